# revision 1
# baseline (speedup 1.0000x reference)
"""Multi-head attention (B=2, S=2048, EMB=1024, 16 heads) on 8 Trainium2 cores.

Sharding: core c -> batch c//4, head-group c%4 (4 heads = 256 projection dims).
Each core computes Q/K/V projections for its head group in transposed layout
(Q^T, K^T with head-dim on partitions; V natural), attention without max
subtraction (scores ~ N(0,1), exp never overflows in fp32), the softmax
denominator via a ones-column appended to V (free inside the ctx matmul), and
a row-parallel partial of the output projection.  The host sums the 4 partials
per batch and adds the output bias (the all-reduce of the row-parallel fc_out
is done at unshard time; no device collectives needed).

All matmuls run as float32r (full-rate on the PE at N>=256).  The BIR verifier
requires fp32r matmul operands to be *produced* as fp32r, so every
matmul-feeding tensor is declared fp32r end-to-end (same 4-byte layout as
fp32; host passes float32 arrays).
"""

import numpy as np

import concourse.tile as tile
from concourse import bacc, mybir
from concourse import bass_utils

EMB = 1024
S = 2048
B = 2
HPC = 4            # heads per core
DQ = HPC * 64      # 256 projection dims per core
NCORES = 8

F32 = mybir.dt.float32
F32R = mybir.dt.float32r
EXP = mybir.ActivationFunctionType.Exp

KT_E = EMB // 128  # 8 contraction tiles over EMB
NQC = S // 512     # 4 query chunks
NST = S // 128     # 16 sequence tiles

_NC = None
TRACE = False
LAST_RESULT = None
_ABLATE = None  # None = all phases; else subset of {"kv", "q", "attn", "out"}
_CTX_PROBE = None
_SPLIT = set()  # heterogeneous in-group splits are rejected by HW; keep off


def _on(phase):
    return _ABLATE is None or phase in _ABLATE


def _mha(ctx, tc, xqT, xkT, xvT, wqT, wkT, wvT, woT, bq, bk, bv, out, bench_iters=None):
    nc = tc.nc

    cpool = ctx.enter_context(tc.tile_pool(name="const", bufs=1))
    xpool = ctx.enter_context(tc.tile_pool(name="xin", bufs=16))
    epool = ctx.enter_context(tc.tile_pool(name="exp", bufs=6))
    bpool = ctx.enter_context(tc.tile_pool(name="bcsb", bufs=2))
    opool = ctx.enter_context(tc.tile_pool(name="osb", bufs=3))
    upool = ctx.enter_context(tc.tile_pool(name="unorm", bufs=8))
    sc_ps = ctx.enter_context(tc.tile_pool(name="scps", bufs=4, space="PSUM"))
    ctx_ps = ctx.enter_context(tc.tile_pool(name="ctxps", bufs=4, space="PSUM"))
    mm_ps = sc_ps
    bc_ps = sc_ps

    # ---- persistent SBUF tensors ----
    ones_row = cpool.tile([1, 512], F32R)
    nc.vector.memset(ones_row[:].bitcast(F32), 1.0)
    sel64 = cpool.tile([65, 64], F32R)          # one-hot: row 64 -> all cols
    nc.vector.memset(sel64[:].bitcast(F32), 0.0)
    nc.vector.memset(sel64[64:65, :].bitcast(F32), 1.0)
    rden = cpool.tile([65, 512], F32R)          # row 64 = denom, rows 0..63 zero
    nc.vector.memset(rden[0:64, :].bitcast(F32), 0.0)

    wq_sb = cpool.tile([128, KT_E * DQ], F32R)  # [128, 2048]: wq_sb[p, n*256+m] = WqT[n*128+p, m]
    wk_sb = cpool.tile([128, KT_E * DQ], F32R)
    wv_sb = cpool.tile([128, KT_E * DQ], F32R)
    for sb, src in ((wq_sb, wqT), (wk_sb, wkT), (wv_sb, wvT)):
        nc.sync.dma_start(
            sb[:].rearrange("p (n m) -> p n m", n=KT_E),
            src.rearrange("(n p) m -> p n m", p=128),
        )
    wo_sb = cpool.tile([128, 2 * EMB], F32R)    # wo_sb[p, n*1024+f] = WoT[n*128+p, f]
    nc.sync.dma_start(
        wo_sb[:].rearrange("p (n m) -> p n m", n=2),
        woT.rearrange("(n p) m -> p n m", p=128),
    )
    bq_sb = cpool.tile([1, DQ], F32R)
    bk_sb = cpool.tile([1, DQ], F32R)
    bv_sb = cpool.tile([1, DQ], F32R)
    for sb, src in ((bq_sb, bq), (bk_sb, bk), (bv_sb, bv)):
        nc.sync.dma_start(sb[:], src[:])

    # results of phase 1/2 kept resident
    kT_sb = cpool.tile([128, 2 * S], F32R)      # [dq-block 2][s 2048]
    qT_sb = cpool.tile([128, 2 * S], F32R)
    ctxT_sb = cpool.tile([128, 2 * S], F32R)
    v_sb = cpool.tile([128, NST * (HPC * 65)], F32R)  # per s-tile: 4 heads x (64 V + ones col)
    nc.vector.memset(
        v_sb[:].bitcast(F32).rearrange("p (t h m) -> p t h m", t=NST, h=HPC)[:, :, :, 64:65],
        1.0,
    )

    if _ABLATE:
        # ablation: un-computed persistent tensors need defined contents
        for t in (kT_sb, qT_sb, ctxT_sb, v_sb):
            nc.vector.memset(t[:].bitcast(F32), 0.001)
        nc.vector.memset(rden[64:65, :].bitcast(F32), 1.0)

    def body():
        _body(tc, nc, xqT, xkT, xvT, out, ones_row, sel64, rden, wq_sb, wk_sb,
              wv_sb, wo_sb, bq_sb, bk_sb, bv_sb, kT_sb, qT_sb, ctxT_sb, v_sb,
              xpool, epool, bpool, opool, upool, mm_ps, sc_ps, ctx_ps, bc_ps)

    if bench_iters:
        hints = (
            mybir.EngineType.PE,
            mybir.EngineType.Activation,
            mybir.EngineType.DVE,
            mybir.EngineType.SP,
            mybir.EngineType.Pool,
        )
        with tc.For_i(0, bench_iters, 1, hint_engines=hints):
            body()
    else:
        body()


def _proj_chain(nc, ps, w_sb, xs, dq, b_sb, ones_row):
    """Q/K projection chain into psum ps: kt=0 full K=128 (group start, marks
    every element written), then K=64 halves on alternating row groups (they
    overlap in the PE array), bias last."""
    col = lambda kt: kt * DQ + dq * 128
    nc.tensor.matmul(ps[:], w_sb[:, col(0): col(0) + 128], xs[0][:],
                     start=True, stop=False)
    for kt in range(1, KT_E):
        if "proj" in _SPLIT:
            for b in (0, 64):
                nc.tensor.matmul(
                    ps[:], w_sb[b:b + 64, col(kt): col(kt) + 128], xs[kt][b:b + 64, :],
                    start=False, stop=False,
                )
        else:
            nc.tensor.matmul(
                ps[:], w_sb[:, col(kt): col(kt) + 128], xs[kt][:],
                start=False, stop=False,
            )
    nc.tensor.matmul(
        ps[:], b_sb[0:1, dq * 128: dq * 128 + 128], ones_row[0:1, :],
        start=False, stop=True,
    )


def _body(tc, nc, xqT, xkT, xvT, out, ones_row, sel64, rden, wq_sb, wk_sb,
          wv_sb, wo_sb, bq_sb, bk_sb, bv_sb, kT_sb, qT_sb, ctxT_sb, v_sb,
          xpool, epool, bpool, opool, upool, mm_ps, sc_ps, ctx_ps, bc_ps):
    pending_norm = []

    def _norm_and_outproj(qc_done):
        _finish_chunk(nc, qc_done, pending_norm, rden, sel64, bc_ps, bpool,
                      opool, mm_ps, ctxT_sb, wo_sb, v_sb, out)

    # ---- phase 1: per sequence chunk, produce Q^T, K^T and V.  Interleaved
    # per chunk so attention on chunk 0 becomes runnable after ~1/4 of the
    # input DMA; the scheduler then fills projection DMA-wait gaps with
    # attention matmuls (and keeps ACT busy with exp) for the rest.
    for qc in range(NQC):
        if _on("q"):
            xq = []
            for kt in range(KT_E):
                t = xpool.tile([128, 512], F32R, tag="xchunk", name=f"xq_{qc}_{kt}")
                nc.sync.dma_start(t[:], xqT[kt, qc])
                xq.append(t)
            for dq in range(2):
                ps = mm_ps.tile([128, 512], F32, tag="sc", name=f"qps_{qc}_{dq}")
                _proj_chain(nc, ps, wq_sb, xq, dq, bq_sb, ones_row)
                nc.vector.tensor_copy(qT_sb[:, dq * S + qc * 512: dq * S + qc * 512 + 512], ps[:])
        if not _on("kv"):
            continue
        xk = []
        for kt in range(KT_E):
            t = xpool.tile([128, 512], F32R, tag="xchunk", name=f"xk_{qc}_{kt}")
            nc.sync.dma_start(t[:], xkT[kt, qc])
            xk.append(t)
        for dq in range(2):
            ps = mm_ps.tile([128, 512], F32, tag="sc", name=f"kps_{qc}_{dq}")
            _proj_chain(nc, ps, wk_sb, xk, dq, bk_sb, ones_row)
            nc.vector.tensor_copy(kT_sb[:, dq * S + qc * 512: dq * S + qc * 512 + 512], ps[:])
        xv = []
        for kt in range(KT_E):
            t = xpool.tile([128, 512], F32R, tag="xchunk", name=f"xv_{qc}_{kt}")
            nc.sync.dma_start(t[:], xvT[kt, qc])
            xv.append(t)
        for sti in range(4):
            st = qc * 4 + sti
            ps = mm_ps.tile([128, 256], F32, tag="sc", name=f"vps_{st}")
            nc.tensor.matmul(
                ps[:], xv[0][:, sti * 128: sti * 128 + 128], wv_sb[:, 0:DQ],
                start=True, stop=False,
            )
            for kt in range(1, KT_E):
                if "v" in _SPLIT:
                    for b in (0, 64):
                        nc.tensor.matmul(
                            ps[:], xv[kt][b:b + 64, sti * 128: sti * 128 + 128],
                            wv_sb[b:b + 64, kt * DQ: kt * DQ + DQ],
                            start=False, stop=False,
                        )
                else:
                    nc.tensor.matmul(
                        ps[:], xv[kt][:, sti * 128: sti * 128 + 128],
                        wv_sb[:, kt * DQ: kt * DQ + DQ],
                        start=False, stop=False,
                    )
            nc.tensor.matmul(
                ps[:], ones_row[0:1, 0:128], bv_sb[0:1, :],
                start=False, stop=True,
            )
            dst = v_sb[:, st * (HPC * 65): (st + 1) * (HPC * 65)]
            nc.vector.tensor_copy(
                dst.rearrange("p (h m) -> p h m", h=HPC)[:, :, 0:64],
                ps[:].rearrange("p (h m) -> p h m", h=HPC),
            )

    # ---- phase 2: per query chunk: attention, then deferred norm/out-proj ----
    for qc in range(NQC):
        # attention, two heads at a time (row-tiled K=64 matmuls can overlap)
        attn_on = _on("attn") or _on("attn_sc") or _on("attn_exp") or _on("attn_ctx")
        do_exp = _on("attn") or _on("attn_exp") or _on("attn_ctx")
        do_ctx = _on("attn") or _on("attn_ctx")
        do_norm = _on("attn")
        for hp in range(HPC // 2 if attn_on else 0):
            # per head: two homogeneous accumulation chains, one per row group
            # (upper 64 k-rows -> bank A at tile row 0, lower -> bank B at row
            # 64); they execute concurrently in the PE array.  Combined at
            # normalize time.
            cps = [
                [
                    ctx_ps.tile([65, 512], F32, tag="ctx", name=f"ctx_{qc}_{hp}_{i}_{half}")
                    for half in range(2)
                ]
                for i in range(2)
            ]

            def ctx_mms(es, kt):
                for hi in range(2):
                    h = hp * 2 + hi
                    rhs = es[hi][:]
                    vcol = kt * (HPC * 65) + h * 65
                    for half, b in enumerate((0, 64)):
                        nc.tensor.matmul(
                            cps[hi][half][:], v_sb[b:b + 64, vcol: vcol + 65],
                            rhs[b:b + 64, :],
                            start=(kt == 0), stop=(kt == NST - 1),
                        )

            prev = None
            for kt in range(NST):
                es = []
                for hi in range(2):
                    base = 64 * hi
                    blk = hp * S
                    sc = sc_ps.tile([128, 512], F32, tag="sc", name=f"sc_{qc}_{hp}_{kt}_{hi}")
                    nc.tensor.matmul(
                        sc[:],
                        kT_sb[base:base + 64, blk + kt * 128: blk + kt * 128 + 128],
                        qT_sb[base:base + 64, blk + qc * 512: blk + qc * 512 + 512],
                        start=True, stop=True,
                    )
                    if do_exp:
                        e = epool.tile([128, 512], F32R, tag="e", name=f"e_{qc}_{hp}_{kt}_{hi}")
                        nc.scalar.activation(e[:], sc[:], EXP, scale=0.125)
                        es.append(e)
                if do_ctx:
                    if prev is not None:
                        ctx_mms(*prev)
                    prev = (es, kt)
            if do_ctx:
                ctx_mms(*prev)
            for hi in range(2 if do_norm else 0):
                # drain PSUM only: combine the two half-chains into SBUF.
                # normalization + out-projection are deferred one qc so they
                # overlap with the next chunk's ACT-bound attention.
                tmpa = bpool.tile([65, 512], F32, tag="tmpa", name=f"tmpa_{qc}_{hp}_{hi}")
                nc.vector.tensor_copy(tmpa[:], cps[hi][0][:])
                u = upool.tile([65, 512], F32, tag="u", name=f"u_{qc}_{hp}_{hi}")
                nc.vector.tensor_add(u[:], cps[hi][1][:], tmpa[:])
                pending_norm.append((qc, hp, hi, u))

        if qc > 0:
            _norm_and_outproj(qc - 1)
    _norm_and_outproj(NQC - 1)


def _finish_chunk(nc, qc, pending_norm, rden, sel64, bc_ps, bpool, opool, mm_ps,
                  ctxT_sb, wo_sb, _unused, out):
    for pqc, hp, hi, u in [p for p in pending_norm if p[0] == qc]:
        # broadcast the raw denominator to 64 partitions via one-hot matmul,
        # then reciprocal + multiply (normalizes ctx into ctxT_sb)
        nc.vector.tensor_copy(rden[64:65, :], u[64:65, :])
        bps = bc_ps.tile([64, 512], F32, tag="sc", name=f"bc_{qc}_{hp}_{hi}")
        nc.tensor.matmul(bps[:], sel64[:], rden[:], start=True, stop=True)
        brec = bpool.tile([64, 512], F32, tag="br", name=f"br_{qc}_{hp}_{hi}")
        nc.vector.reciprocal(brec[:], bps[:])
        nc.vector.tensor_mul(
            ctxT_sb[64 * hi: 64 * hi + 64, hp * S + qc * 512: hp * S + qc * 512 + 512],
            u[0:64, :],
            brec[:],
        )
    if not _on("out"):
        return
    # out projection for this chunk's 4 query tiles
    for qt4 in range(4):
        qt = qc * 4 + qt4
        ot = opool.tile([128, EMB], F32, tag="o", name=f"ot_{qt}")
        for fc in range(2):
            ps = mm_ps.tile([128, 512], F32, tag="sc", name=f"ops_{qt}_{fc}")
            nc.tensor.matmul(
                ps[:],
                ctxT_sb[:, qt * 128: qt * 128 + 128],
                wo_sb[:, fc * 512: fc * 512 + 512],
                start=True, stop=False,
            )
            nc.tensor.matmul(
                ps[:],
                ctxT_sb[:, S + qt * 128: S + qt * 128 + 128],
                wo_sb[:, EMB + fc * 512: EMB + fc * 512 + 512],
                start=False, stop=True,
            )
            nc.vector.tensor_copy(ot[:, fc * 512: fc * 512 + 512], ps[:])
        nc.gpsimd.dma_start(out[qt * 128:(qt + 1) * 128, :], ot[:])


def _build_nc(bench_iters=None):
    from contextlib import ExitStack

    nc = bacc.Bacc("TRN2", target_bir_lowering=False, debug=False, num_devices=NCORES)
    xqT = nc.dram_tensor("xqT", [KT_E, NQC, 128, 512], F32R, kind="ExternalInput").ap()
    xkT = nc.dram_tensor("xkT", [KT_E, NQC, 128, 512], F32R, kind="ExternalInput").ap()
    xvT = nc.dram_tensor("xvT", [KT_E, NQC, 128, 512], F32R, kind="ExternalInput").ap()
    wqT = nc.dram_tensor("wqT", [EMB, DQ], F32R, kind="ExternalInput").ap()
    wkT = nc.dram_tensor("wkT", [EMB, DQ], F32R, kind="ExternalInput").ap()
    wvT = nc.dram_tensor("wvT", [EMB, DQ], F32R, kind="ExternalInput").ap()
    woT = nc.dram_tensor("woT", [DQ, EMB], F32R, kind="ExternalInput").ap()
    bq = nc.dram_tensor("bq", [1, DQ], F32R, kind="ExternalInput").ap()
    bk = nc.dram_tensor("bk", [1, DQ], F32R, kind="ExternalInput").ap()
    bv = nc.dram_tensor("bv", [1, DQ], F32R, kind="ExternalInput").ap()
    out = nc.dram_tensor("out", [S, EMB], F32, kind="ExternalOutput").ap()

    with ExitStack() as ctx:
        tc = ctx.enter_context(tile.TileContext(nc))
        _mha(ctx, tc, xqT, xkT, xvT, wqT, wkT, wvT, woT, bq, bk, bv, out,
             bench_iters=bench_iters)
    nc.compile()
    return nc


def _chunk_major(x):
    """[S, EMB] -> x.T chunked as [KT_E, NQC, 128, 512] (each chunk contiguous)."""
    xt = x.T  # [EMB, S]
    return np.ascontiguousarray(
        xt.reshape(KT_E, 128, NQC, 512).transpose(0, 2, 1, 3)
    )


def kernel(query, key, value, Wq, bq, Wk, bk, Wv, bv, Wo, bo):
    global _NC, LAST_RESULT
    query, key, value, Wq, bq, Wk, bk, Wv, bv, Wo, bo = (
        np.asarray(a, dtype=np.float32)
        for a in (query, key, value, Wq, bq, Wk, bk, Wv, bv, Wo, bo)
    )
    if _NC is None:
        _NC = _build_nc()

    in_maps = []
    for c in range(NCORES):
        b, g = divmod(c, 4)
        rows = slice(g * DQ, (g + 1) * DQ)
        in_maps.append({
            "xqT": _chunk_major(query[b]),
            "xkT": _chunk_major(key[b]),
            "xvT": _chunk_major(value[b]),
            "wqT": np.ascontiguousarray(Wq[rows].T),
            "wkT": np.ascontiguousarray(Wk[rows].T),
            "wvT": np.ascontiguousarray(Wv[rows].T),
            "woT": np.ascontiguousarray(Wo[:, rows].T),
            "bq": np.ascontiguousarray(bq[rows][None, :]),
            "bk": np.ascontiguousarray(bk[rows][None, :]),
            "bv": np.ascontiguousarray(bv[rows][None, :]),
        })

    res = bass_utils.run_bass_kernel_spmd(
        _NC, in_maps, core_ids=list(range(NCORES)), trace=TRACE
    )
    LAST_RESULT = res

    out = np.zeros((B, S, EMB), np.float32)
    for c in range(NCORES):
        out[c // 4] += res.results[c]["out"]
    out += bo[None, None, :]
    return out



# revision 8
# speedup vs baseline: 1.0437x; 1.0437x over previous
"""Multi-head attention (B=2, S=2048, EMB=1024, 16 heads) on 8 Trainium2 cores.

Sharding: core c -> batch c//4, head-group c%4 (4 heads = 256 projection dims).
Each core computes Q/K/V projections for its head group in transposed layout
(Q^T, K^T with head-dim on partitions; V natural), attention without max
subtraction (scores ~ N(0,1), exp never overflows in fp32), and a row-parallel
partial of the output projection.  The host sums the 4 partials per batch and
adds the output bias (the all-reduce of the row-parallel fc_out is done at
unshard time; no device collectives needed).

Key PE-efficiency choices (PE issue time = moving-dim cycles per matmul,
independent of contraction rows):
  - ctx accumulation runs one K=128 chain per head (not two K=64 halves).
  - The stationary V block per (k-tile, head) is [128, 128]: cols 0..63 the
    head's V, cols 64..127 all-ones.  Out rows 64..127 then all hold the
    softmax denominator -- replicated across partitions for free by the PE
    (those array columns were idle anyway), so normalization is just a
    reciprocal + multiply on DVE with no broadcast step.
  - Q/K biases are folded into the PSUM->SBUF drain (tensor_scalar_add);
    V bias stays as a 1-row matmul (free-dim bias can't ride the drain).
  - exp runs one ACT instruction per [128, 1024] PSUM tile (a head-pair's
    scores for one k-tile), halving ACT per-instruction overhead.
  - K/V projections for all chunks run first, then Q + attention per query
    chunk, so the ACT exp stream starts as early as possible.

All matmuls run as float32r (full-rate on the PE at N>=256).  The BIR verifier
requires fp32r matmul operands to be *produced* as fp32r, so every
matmul-feeding tensor is declared fp32r end-to-end (same 4-byte layout as
fp32; host passes float32 arrays).
"""

import ml_dtypes
import numpy as np

import concourse.tile as tile
from concourse import bacc, mybir
from concourse import bass_utils

EMB = 1024
S = 2048
B = 2
HPC = 4            # heads per core
DQ = HPC * 64      # 256 projection dims per core
NCORES = 8

F32 = mybir.dt.float32
F32R = mybir.dt.float32r
BF16 = mybir.dt.bfloat16
EXP = mybir.ActivationFunctionType.Exp

KT_E = EMB // 128  # 8 contraction tiles over EMB
NQC = S // 512     # 4 query chunks
NST = S // 128     # 16 sequence tiles

_NC = None
TRACE = False
LAST_RESULT = None


def _mha(ctx, tc, xqT, xkT, xvT, wqT, wkT, wvT, woT, bq, bk, bv, out, bench_iters=None):
    nc = tc.nc

    cpool = ctx.enter_context(tc.tile_pool(name="const", bufs=1))
    xpool = ctx.enter_context(tc.tile_pool(name="xin", bufs=4))
    epool = ctx.enter_context(tc.tile_pool(name="exp", bufs=4))
    npool = ctx.enter_context(tc.tile_pool(name="norm", bufs=4))
    opool = ctx.enter_context(tc.tile_pool(name="osb", bufs=3))
    qk_ps = ctx.enter_context(tc.tile_pool(name="qkps", bufs=2, space="PSUM"))
    sc_ps = ctx.enter_context(tc.tile_pool(name="scps", bufs=2, space="PSUM"))
    ctx_ps = ctx.enter_context(tc.tile_pool(name="ctxps", bufs=2, space="PSUM"))

    # ---- persistent SBUF tensors ----
    # K/V weights first: phase 1 needs them before anything else.
    ones_row = cpool.tile([1, 512], BF16)
    nc.vector.memset(ones_row[:], 1.0)

    wq_sb = cpool.tile([128, KT_E * DQ], BF16)  # [128, 2048]: wq_sb[p, n*256+m] = WqT[n*128+p, m]
    wk_sb = cpool.tile([128, KT_E * DQ], BF16)
    wv_sb = cpool.tile([128, KT_E * DQ], BF16)
    for sb, src in ((wk_sb, wkT), (wv_sb, wvT), (wq_sb, wqT)):
        nc.scalar.dma_start(
            sb[:].rearrange("p (n m) -> p n m", n=KT_E),
            src.rearrange("(n p) m -> p n m", p=128),
        )
    bq_sb = cpool.tile([128, 2], F32)           # bq_sb[p, dq] = bq[dq*128+p]
    bk_sb = cpool.tile([128, 2], F32)
    for sb, src in ((bq_sb, bq), (bk_sb, bk)):
        nc.scalar.dma_start(sb[:], src[:])
    bv_sb = cpool.tile([1, DQ], BF16)
    nc.scalar.dma_start(bv_sb[:], bv[:])
    wo_sb = cpool.tile([128, 2 * EMB], F32R)    # wo_sb[p, n*1024+f] = WoT[n*128+p, f]
    nc.scalar.dma_start(
        wo_sb[:].rearrange("p (n m) -> p n m", n=2),
        woT.rearrange("(n p) m -> p n m", p=128),
    )

    # results of the projection phases kept resident
    kT_sb = cpool.tile([128, 2 * S], F32R)      # [dq-block 2][s 2048]
    qT_sb = cpool.tile([128, 2 * S], F32R)
    ctxT_sb = cpool.tile([128, 2 * S], F32R)
    # per (k-tile, head) block of 128 cols: [0:64] = V, [64:128] = ones
    v_sb = cpool.tile([128, NST * (HPC * 128)], F32R)
    nc.vector.memset(
        v_sb[:].bitcast(F32).rearrange("p (t h m) -> p t h m", t=NST, h=HPC)[:, :, :, 64:128],
        1.0,
    )

    def body():
        _body(tc, nc, xqT, xkT, xvT, out, ones_row, wq_sb, wk_sb,
              wv_sb, wo_sb, bq_sb, bk_sb, bv_sb, kT_sb, qT_sb, ctxT_sb, v_sb,
              xpool, epool, npool, opool, qk_ps, sc_ps, ctx_ps)

    if bench_iters:
        hints = (
            mybir.EngineType.PE,
            mybir.EngineType.Activation,
            mybir.EngineType.DVE,
            mybir.EngineType.SP,
            mybir.EngineType.Pool,
        )
        with tc.For_i(0, bench_iters, 1, hint_engines=hints):
            body()
    else:
        body()


def _proj_chain(nc, ps, w_sb, xs, dq):
    """Q/K projection chain into psum ps: kt=0 full K=128 (group start, marks
    every element written), then 7 more accumulating K=128 steps."""
    col = lambda kt: kt * DQ + dq * 128
    for kt in range(KT_E):
        nc.tensor.matmul(
            ps[:], w_sb[:, col(kt): col(kt) + 128], xs[kt],
            start=(kt == 0), stop=(kt == KT_E - 1),
        )


def _body(tc, nc, xqT, xkT, xvT, out, ones_row, wq_sb, wk_sb,
          wv_sb, wo_sb, bq_sb, bk_sb, bv_sb, kT_sb, qT_sb, ctxT_sb, v_sb,
          xpool, epool, npool, opool, qk_ps, sc_ps, ctx_ps):
    # ---- phase 1: K and V projections for every chunk (Q deferred so the
    # attention pipeline -- and with it the ACT exp stream -- starts as soon
    # as K/V are resident).
    for qc in range(NQC):
        xkb = xpool.tile([128, KT_E * 512], BF16, tag="xchunk", name=f"xk_{qc}")
        nc.sync.dma_start(xkb[:], xkT[qc])
        xk = [xkb[:, kt * 512: kt * 512 + 512] for kt in range(KT_E)]
        for dq in range(2):
            ps = qk_ps.tile([128, 512], F32, tag="qk", name=f"kps_{qc}_{dq}")
            _proj_chain(nc, ps, wk_sb, xk, dq)
            nc.vector.tensor_scalar_add(
                kT_sb[:, dq * S + qc * 512: dq * S + qc * 512 + 512],
                ps[:], bk_sb[:, dq: dq + 1],
            )
        xvb = xpool.tile([128, KT_E * 512], BF16, tag="xchunk", name=f"xv_{qc}")
        nc.sync.dma_start(xvb[:], xvT[qc])
        xv = [xvb[:, kt * 512: kt * 512 + 512] for kt in range(KT_E)]
        for sti in range(4):
            st = qc * 4 + sti
            ps = sc_ps.tile([128, 256], F32, tag="sc", name=f"vps_{st}")
            for kt in range(KT_E):
                nc.tensor.matmul(
                    ps[:], xv[kt][:, sti * 128: sti * 128 + 128],
                    wv_sb[:, kt * DQ: kt * DQ + DQ],
                    start=(kt == 0), stop=False,
                )
            nc.tensor.matmul(
                ps[:], ones_row[0:1, 0:128], bv_sb[0:1, :],
                start=False, stop=True,
            )
            dst = v_sb[:, st * (HPC * 128): (st + 1) * (HPC * 128)]
            nc.vector.tensor_copy(
                dst.rearrange("p (h m) -> p h m", h=HPC)[:, :, 0:64],
                ps[:].rearrange("p (h m) -> p h m", h=HPC),
            )

    # ---- phase 2: per query chunk: Q projection, attention, then the
    # previous chunk's output projection (deferred one qc so it overlaps
    # with this chunk's ACT-bound attention).
    for qc in range(NQC):
        xqb = xpool.tile([128, KT_E * 512], BF16, tag="xchunk", name=f"xq_{qc}")
        nc.sync.dma_start(xqb[:], xqT[qc])
        xq = [xqb[:, kt * 512: kt * 512 + 512] for kt in range(KT_E)]
        for dq in range(2):
            ps = qk_ps.tile([128, 512], F32, tag="qk", name=f"qps_{qc}_{dq}")
            _proj_chain(nc, ps, wq_sb, xq, dq)
            nc.vector.tensor_scalar_add(
                qT_sb[:, dq * S + qc * 512: dq * S + qc * 512 + 512],
                ps[:], bq_sb[:, dq: dq + 1],
            )

        # attention, two heads at a time (the head pair shares one
        # [128, 1024] score tile -> one exp instruction per k-tile)
        for hp in range(HPC // 2):
            cps = [
                ctx_ps.tile([128, 512], F32, tag="ctx", name=f"ctx_{qc}_{hp}_{hi}")
                for hi in range(2)
            ]

            def ctx_mms(e, kt):
                for hi in range(2):
                    h = hp * 2 + hi
                    vcol = (kt * HPC + h) * 128
                    nc.tensor.matmul(
                        cps[hi][:], v_sb[:, vcol: vcol + 128],
                        e[:, hi * 512: hi * 512 + 512],
                        start=(kt == 0), stop=(kt == NST - 1),
                    )

            prev = None
            for kt in range(NST):
                blk = hp * S
                sc = sc_ps.tile([128, 1024], F32, tag="sc", name=f"sc_{qc}_{hp}_{kt}")
                for hi in range(2):
                    base = 64 * hi
                    nc.tensor.matmul(
                        sc[:, hi * 512: hi * 512 + 512],
                        kT_sb[base:base + 64, blk + kt * 128: blk + kt * 128 + 128],
                        qT_sb[base:base + 64, blk + qc * 512: blk + qc * 512 + 512],
                        start=True, stop=True,
                    )
                e = epool.tile([128, 1024], F32R, tag="e", name=f"e_{qc}_{hp}_{kt}")
                nc.scalar.activation(e[:], sc[:], EXP, scale=0.125)
                if prev is not None:
                    ctx_mms(*prev)
                prev = (e, kt)
            ctx_mms(*prev)

            # normalize: rows 64..127 of each ctx psum tile hold the softmax
            # denominator (replicated by the all-ones V columns)
            for hi in range(2):
                rec = npool.tile([64, 512], F32, tag="rec", name=f"rec_{qc}_{hp}_{hi}")
                nc.vector.reciprocal(rec[:], cps[hi][64:128, :])
                nc.vector.tensor_mul(
                    ctxT_sb[64 * hi: 64 * hi + 64, hp * S + qc * 512: hp * S + qc * 512 + 512],
                    cps[hi][0:64, :],
                    rec[:],
                )

        if qc > 0:
            _out_proj(nc, qc - 1, qk_ps, opool, ctxT_sb, wo_sb, out)
    _out_proj(nc, NQC - 1, qk_ps, opool, ctxT_sb, wo_sb, out)


def _out_proj(nc, qc, qk_ps, opool, ctxT_sb, wo_sb, out):
    # out projection for this chunk's 4 query tiles
    for qt4 in range(4):
        qt = qc * 4 + qt4
        ot = opool.tile([128, EMB], F32, tag="o", name=f"ot_{qt}")
        for fc in range(2):
            ps = qk_ps.tile([128, 512], F32, tag="qk", name=f"ops_{qt}_{fc}")
            nc.tensor.matmul(
                ps[:],
                ctxT_sb[:, qt * 128: qt * 128 + 128],
                wo_sb[:, fc * 512: fc * 512 + 512],
                start=True, stop=False,
            )
            nc.tensor.matmul(
                ps[:],
                ctxT_sb[:, S + qt * 128: S + qt * 128 + 128],
                wo_sb[:, EMB + fc * 512: EMB + fc * 512 + 512],
                start=False, stop=True,
            )
            nc.vector.tensor_copy(ot[:, fc * 512: fc * 512 + 512], ps[:])
        nc.gpsimd.dma_start(out[qt * 128:(qt + 1) * 128, :], ot[:])


def _build_nc(bench_iters=None):
    from contextlib import ExitStack

    nc = bacc.Bacc("TRN2", target_bir_lowering=False, debug=False, num_devices=NCORES)
    xqT = nc.dram_tensor("xqT", [NQC, 128, KT_E * 512], BF16, kind="ExternalInput").ap()
    xkT = nc.dram_tensor("xkT", [NQC, 128, KT_E * 512], BF16, kind="ExternalInput").ap()
    xvT = nc.dram_tensor("xvT", [NQC, 128, KT_E * 512], BF16, kind="ExternalInput").ap()
    wqT = nc.dram_tensor("wqT", [EMB, DQ], BF16, kind="ExternalInput").ap()
    wkT = nc.dram_tensor("wkT", [EMB, DQ], BF16, kind="ExternalInput").ap()
    wvT = nc.dram_tensor("wvT", [EMB, DQ], BF16, kind="ExternalInput").ap()
    woT = nc.dram_tensor("woT", [DQ, EMB], F32R, kind="ExternalInput").ap()
    bq = nc.dram_tensor("bq", [128, 2], F32, kind="ExternalInput").ap()
    bk = nc.dram_tensor("bk", [128, 2], F32, kind="ExternalInput").ap()
    bv = nc.dram_tensor("bv", [1, DQ], BF16, kind="ExternalInput").ap()
    out = nc.dram_tensor("out", [S, EMB], F32, kind="ExternalOutput").ap()

    with ExitStack() as ctx:
        tc = ctx.enter_context(tile.TileContext(nc))
        _mha(ctx, tc, xqT, xkT, xvT, wqT, wkT, wvT, woT, bq, bk, bv, out,
             bench_iters=bench_iters)
    nc.compile()
    return nc


def _chunk_major(x):
    """[S, EMB] -> x.T as [NQC, 128, KT_E*512]: arr[qc, p, kt*512+m] =
    x.T[kt*128+p, qc*512+m] (one contiguous [128, 4096] DMA per chunk)."""
    xt = np.asarray(x, np.float32).T  # [EMB, S]
    arr = xt.reshape(KT_E, 128, NQC, 512).transpose(2, 1, 0, 3).reshape(NQC, 128, KT_E * 512)
    return np.ascontiguousarray(arr.astype(ml_dtypes.bfloat16))


def make_in_maps(query, key, value, Wq, bq, Wk, bk, Wv, bv, Wo, bo):
    in_maps = []
    for c in range(NCORES):
        b, g = divmod(c, 4)
        rows = slice(g * DQ, (g + 1) * DQ)
        in_maps.append({
            "xqT": _chunk_major(np.asarray(query, np.float32)[b]),
            "xkT": _chunk_major(np.asarray(key, np.float32)[b]),
            "xvT": _chunk_major(np.asarray(value, np.float32)[b]),
            "wqT": np.ascontiguousarray(np.asarray(Wq, np.float32)[rows].T.astype(ml_dtypes.bfloat16)),
            "wkT": np.ascontiguousarray(np.asarray(Wk, np.float32)[rows].T.astype(ml_dtypes.bfloat16)),
            "wvT": np.ascontiguousarray(np.asarray(Wv, np.float32)[rows].T.astype(ml_dtypes.bfloat16)),
            "woT": np.ascontiguousarray(np.asarray(Wo, np.float32)[:, rows].T),
            "bq": np.ascontiguousarray(np.asarray(bq, np.float32)[rows].reshape(2, 128).T),
            "bk": np.ascontiguousarray(np.asarray(bk, np.float32)[rows].reshape(2, 128).T),
            "bv": np.ascontiguousarray(np.asarray(bv, np.float32)[rows][None, :].astype(ml_dtypes.bfloat16)),
        })
    return in_maps


def kernel(query, key, value, Wq, bq, Wk, bk, Wv, bv, Wo, bo):
    global _NC, LAST_RESULT
    bo = np.asarray(bo, dtype=np.float32)
    if _NC is None:
        _NC = _build_nc()

    in_maps = make_in_maps(query, key, value, Wq, bq, Wk, bk, Wv, bv, Wo, bo)

    res = bass_utils.run_bass_kernel_spmd(
        _NC, in_maps, core_ids=list(range(NCORES)), trace=TRACE
    )
    LAST_RESULT = res

    out = np.zeros((B, S, EMB), np.float32)
    for c in range(NCORES):
        out[c // 4] += res.results[c]["out"]
    out += bo[None, None, :]
    return out


# revision 20
# speedup vs baseline: 1.2066x; 1.1560x over previous
"""Multi-head attention (B=2, S=2048, EMB=1024, 16 heads) on 8 Trainium2 cores.

Sharding: core c -> batch c//4, head-group c%4 (4 heads = 256 projection dims).
Each core computes Q/K/V projections for its head group in transposed layout
(Q^T, K^T with head-dim on partitions; V natural), attention without max
subtraction (scores ~ N(0,1), exp never overflows in fp32), and a row-parallel
partial of the output projection.  The host sums the 4 partials per batch and
adds the output bias (the all-reduce of the row-parallel fc_out is done at
unshard time; no device collectives needed).

Key PE-efficiency choices (PE issue time = moving-dim cycles per matmul,
independent of contraction rows):
  - ctx accumulation runs one K=128 chain per head (not two K=64 halves).
  - The stationary V block per (k-tile, head) is [128, 128]: cols 0..63 the
    head's V, cols 64..127 all-ones.  Out rows 64..127 then all hold the
    softmax denominator -- replicated across partitions for free by the PE
    (those array columns were idle anyway), so normalization is just a
    reciprocal + multiply on DVE with no broadcast step.
  - Q/K biases are folded into the PSUM->SBUF drain (tensor_scalar_add);
    V bias stays as a 1-row matmul (free-dim bias can't ride the drain).
  - exp runs one ACT instruction per [128, 1024] PSUM tile (a head-pair's
    scores for one k-tile), halving ACT per-instruction overhead.
  - K/V projections for all chunks run first, then Q + attention per query
    chunk, so the ACT exp stream starts as early as possible.
  - Attention passes are zigzag-pipelined: pass p's scores+exp (PE->ACT) run
    concurrently with pass p-1's ctx chains, whose es inputs (bf16, SBUF ring
    across two passes) are long since ready -- the PE never waits on ACT.

All matmuls run as float32r (full-rate on the PE at N>=256).  The BIR verifier
requires fp32r matmul operands to be *produced* as fp32r, so every
matmul-feeding tensor is declared fp32r end-to-end (same 4-byte layout as
fp32; host passes float32 arrays).
"""

import ml_dtypes
import numpy as np

import concourse.tile as tile
from concourse import bacc, mybir
from concourse import bass_utils

EMB = 1024
S = 2048
B = 2
HPC = 4            # heads per core
DQ = HPC * 64      # 256 projection dims per core
NCORES = 8

F32 = mybir.dt.float32
F32R = mybir.dt.float32r
BF16 = mybir.dt.bfloat16
EXP = mybir.ActivationFunctionType.Exp

KT_E = EMB // 128  # 8 contraction tiles over EMB
NQC = S // 512     # 4 query chunks
NST = S // 128     # 16 sequence tiles

_NC = None
TRACE = False
LAST_RESULT = None
_SMALL_DMA = False
_PHASES = None   # None = all; else subset of {"proj","sc","exp","ctx","out"}
_SAME_KT = False # bench probe: all scores matmuls reuse one stationary block
_CTX_BF16 = False  # es + v in bf16 (correct kernel; probes fp32r accum rate)
_CTX_NODEP = False # bench probe: ctx rhs is a constant tile (no ACT dep)
_CTX_ALLSTART = False  # bench probe: ctx matmuls all start/stop (no accumulate)
_CTX_HALVES = False    # ctx as two row-disjoint K=64 half chains (correct)
_ZIGZAG = True   # ctx chains of pass p-1 run against scores+exp of pass p


def _on(ph):
    return _PHASES is None or ph in _PHASES


def _mha(ctx, tc, xqT, xkT, xvT, wqT, wkT, wvT, woT, bq, bk, bv, out, bench_iters=None):
    nc = tc.nc

    cpool = ctx.enter_context(tc.tile_pool(name="const", bufs=1))
    xpool = ctx.enter_context(tc.tile_pool(name="xin", bufs=4))
    epool = ctx.enter_context(tc.tile_pool(name="exp", bufs=34 if _ZIGZAG else 4))
    npool = ctx.enter_context(tc.tile_pool(name="norm", bufs=4))
    opool = ctx.enter_context(tc.tile_pool(name="osb", bufs=3))
    qk_ps = ctx.enter_context(tc.tile_pool(name="qkps", bufs=1 if _CTX_HALVES else 2, space="PSUM"))
    sc_ps = ctx.enter_context(tc.tile_pool(name="scps", bufs=3 if _CTX_HALVES else 2, space="PSUM"))
    ctx_ps = ctx.enter_context(tc.tile_pool(name="ctxps", bufs=4 if _CTX_HALVES else 2, space="PSUM"))

    # ---- persistent SBUF tensors ----
    # K/V weights first: phase 1 needs them before anything else.
    ones_row = cpool.tile([1, 512], BF16)
    nc.vector.memset(ones_row[:], 1.0)

    wq_sb = cpool.tile([128, KT_E * DQ], BF16)  # [128, 2048]: wq_sb[p, n*256+m] = WqT[n*128+p, m]
    wk_sb = cpool.tile([128, KT_E * DQ], BF16)
    wv_sb = cpool.tile([128, KT_E * DQ], BF16)
    for sb, src in ((wk_sb, wkT), (wv_sb, wvT), (wq_sb, wqT)):
        nc.scalar.dma_start(
            sb[:].rearrange("p (n m) -> p n m", n=KT_E),
            src.rearrange("(n p) m -> p n m", p=128),
        )
    bq_sb = cpool.tile([128, 2], F32)           # bq_sb[p, dq] = bq[dq*128+p]
    bk_sb = cpool.tile([128, 2], F32)
    for sb, src in ((bq_sb, bq), (bk_sb, bk)):
        nc.scalar.dma_start(sb[:], src[:])
    bv_sb = cpool.tile([1, DQ], BF16)
    nc.scalar.dma_start(bv_sb[:], bv[:])
    wo_sb = cpool.tile([128, 2 * EMB], F32R)    # wo_sb[p, n*1024+f] = WoT[n*128+p, f]
    nc.scalar.dma_start(
        wo_sb[:].rearrange("p (n m) -> p n m", n=2),
        woT.rearrange("(n p) m -> p n m", p=128),
    )

    # results of the projection phases kept resident
    kT_sb = cpool.tile([128, 2 * S], F32R)      # [dq-block 2][s 2048]
    qT_sb = cpool.tile([128, 2 * S], F32R)
    ctxT_sb = cpool.tile([128, 2 * S], F32R)
    # per (k-tile, head) block of 128 cols: [0:64] = V, [64:128] = ones
    v_dt = BF16 if (_CTX_BF16 or _ZIGZAG) else F32R
    v_sb = cpool.tile([128, NST * (HPC * 128)], v_dt)
    v_ones = v_sb[:].rearrange("p (t h m) -> p t h m", t=NST, h=HPC)[:, :, :, 64:128]
    nc.vector.memset(v_ones if v_dt == BF16 else v_ones.bitcast(F32), 1.0)

    dummies = {}
    if _PHASES is not None:
        for t in (kT_sb, qT_sb, ctxT_sb, v_sb):
            nc.vector.memset(t[:].bitcast(F32), 0.001)
        if not _on("sc"):
            dsc = sc_ps.tile([128, 1024], F32, tag="sc", name="dummy_sc")
            nc.vector.memset(dsc[:], 0.001)
            dummies["sc"] = dsc
        if not _on("exp"):
            de = epool.tile([128, 1024], F32R, tag="e", name="dummy_e")
            nc.vector.memset(de[:].bitcast(F32), 0.001)
            dummies["e"] = de
    if _CTX_NODEP:
        dn = epool.tile([128, 1024], BF16 if _CTX_BF16 else F32R, tag="e", name="dummy_nodep")
        nc.vector.memset(dn[:] if _CTX_BF16 else dn[:].bitcast(F32), 0.001)
        dummies["nodep"] = dn

    def body():
        _body(tc, nc, xqT, xkT, xvT, out, ones_row, wq_sb, wk_sb,
              wv_sb, wo_sb, bq_sb, bk_sb, bv_sb, kT_sb, qT_sb, ctxT_sb, v_sb,
              xpool, epool, npool, opool, qk_ps, sc_ps, ctx_ps, dummies)

    if bench_iters:
        hints = (
            mybir.EngineType.PE,
            mybir.EngineType.Activation,
            mybir.EngineType.DVE,
            mybir.EngineType.SP,
            mybir.EngineType.Pool,
        )
        with tc.For_i(0, bench_iters, 1, hint_engines=hints):
            body()
    else:
        body()


class _HalvesE:
    def __init__(self, es):
        self.es = es

    def __getitem__(self, idx):
        rows, cols = idx
        hi = cols.start // 512
        off = cols.start - hi * 512
        return self.es[hi][rows, off: off + (cols.stop - cols.start)]


def _proj_chain(nc, ps, w_sb, xs, dq):
    """Q/K projection chain into psum ps: kt=0 full K=128 (group start, marks
    every element written), then 7 more accumulating K=128 steps."""
    col = lambda kt: kt * DQ + dq * 128
    for kt in range(KT_E):
        nc.tensor.matmul(
            ps[:], w_sb[:, col(kt): col(kt) + 128], xs[kt],
            start=(kt == 0), stop=(kt == KT_E - 1),
        )


def _body(tc, nc, xqT, xkT, xvT, out, ones_row, wq_sb, wk_sb,
          wv_sb, wo_sb, bq_sb, bk_sb, bv_sb, kT_sb, qT_sb, ctxT_sb, v_sb,
          xpool, epool, npool, opool, qk_ps, sc_ps, ctx_ps, dummies=None):
    # ---- phase 1: K and V projections for every chunk (Q deferred so the
    # attention pipeline -- and with it the ACT exp stream -- starts as soon
    # as K/V are resident).
    for qc in range(NQC if _on("proj") else 0):
        xkb = xpool.tile([128, KT_E * 512], BF16, tag="xchunk", name=f"xk_{qc}")
        if _SMALL_DMA:
            for kt in range(KT_E):
                nc.sync.dma_start(xkb[:, kt * 512: kt * 512 + 512],
                                  xkT[qc][:, kt * 512: kt * 512 + 512])
        else:
            nc.sync.dma_start(xkb[:], xkT[qc])
        xk = [xkb[:, kt * 512: kt * 512 + 512] for kt in range(KT_E)]
        for dq in range(2):
            ps = qk_ps.tile([128, 512], F32, tag="qk", name=f"kps_{qc}_{dq}")
            _proj_chain(nc, ps, wk_sb, xk, dq)
            nc.vector.tensor_scalar_add(
                kT_sb[:, dq * S + qc * 512: dq * S + qc * 512 + 512],
                ps[:], bk_sb[:, dq: dq + 1],
            )
        xvb = xpool.tile([128, KT_E * 512], BF16, tag="xchunk", name=f"xv_{qc}")
        if _SMALL_DMA:
            for kt in range(KT_E):
                nc.sync.dma_start(xvb[:, kt * 512: kt * 512 + 512],
                                  xvT[qc][:, kt * 512: kt * 512 + 512])
        else:
            nc.sync.dma_start(xvb[:], xvT[qc])
        xv = [xvb[:, kt * 512: kt * 512 + 512] for kt in range(KT_E)]
        for sti in range(4):
            st = qc * 4 + sti
            if _CTX_HALVES:
                ps = qk_ps.tile([128, 256], F32, tag="qk", name=f"vps_{st}")
            else:
                ps = sc_ps.tile([128, 256], F32, tag="sc", name=f"vps_{st}")
            for kt in range(KT_E):
                nc.tensor.matmul(
                    ps[:], xv[kt][:, sti * 128: sti * 128 + 128],
                    wv_sb[:, kt * DQ: kt * DQ + DQ],
                    start=(kt == 0), stop=False,
                )
            nc.tensor.matmul(
                ps[:], ones_row[0:1, 0:128], bv_sb[0:1, :],
                start=False, stop=True,
            )
            dst = v_sb[:, st * (HPC * 128): (st + 1) * (HPC * 128)]
            nc.vector.tensor_copy(
                dst.rearrange("p (h m) -> p h m", h=HPC)[:, :, 0:64],
                ps[:].rearrange("p (h m) -> p h m", h=HPC),
            )

    # ---- phase 2: per query chunk: Q projection, attention, then the
    # previous chunk's output projection (deferred one qc so it overlaps
    # with this chunk's ACT-bound attention).
    zig_state = {"prev": None}
    for qc in range(NQC):
      if _on("proj"):
        xqb = xpool.tile([128, KT_E * 512], BF16, tag="xchunk", name=f"xq_{qc}")
        if _SMALL_DMA:
            for kt in range(KT_E):
                nc.sync.dma_start(xqb[:, kt * 512: kt * 512 + 512],
                                  xqT[qc][:, kt * 512: kt * 512 + 512])
        else:
            nc.sync.dma_start(xqb[:], xqT[qc])
        xq = [xqb[:, kt * 512: kt * 512 + 512] for kt in range(KT_E)]
        for dq in range(2):
            ps = qk_ps.tile([128, 512], F32, tag="qk", name=f"qps_{qc}_{dq}")
            _proj_chain(nc, ps, wq_sb, xq, dq)
            nc.vector.tensor_scalar_add(
                qT_sb[:, dq * S + qc * 512: dq * S + qc * 512 + 512],
                ps[:], bq_sb[:, dq: dq + 1],
            )

        # attention, two heads at a time (the head pair shares one
        # [128, 1024] score tile -> one exp instruction per k-tile)
        for hp in range(HPC // 2):
            if _ZIGZAG:
                _zig_pass(nc, qc, hp, kT_sb, qT_sb, ctxT_sb, v_sb, sc_ps, ctx_ps,
                          epool, npool, zig_state)
                continue
            if _on("ctx") and _CTX_HALVES:
                cps = [
                    [ctx_ps.tile([128, 512], F32, tag="ctx", name=f"ctx_{qc}_{hp}_{hi}_{ha}")
                     for ha in range(2)]
                    for hi in range(2)
                ]
            elif _on("ctx"):
                cps = [
                    ctx_ps.tile([128, 512], F32, tag="ctx", name=f"ctx_{qc}_{hp}_{hi}")
                    for hi in range(2)
                ]
            else:
                cps = None

            def ctx_mms(e, kt):
                for hi in range(2):
                    h = hp * 2 + hi
                    vcol = (kt * HPC + h) * 128
                    if _CTX_HALVES:
                        for ha, b in enumerate((0, 64)):
                            nc.tensor.matmul(
                                cps[hi][ha][:], v_sb[b:b + 64, vcol: vcol + 128],
                                e[b:b + 64, hi * 512: hi * 512 + 512],
                                start=(kt == 0), stop=(kt == NST - 1),
                            )
                    elif _CTX_ALLSTART:
                        nc.tensor.matmul(
                            cps[hi][:], v_sb[:, vcol: vcol + 128],
                            e[:, hi * 512: hi * 512 + 512],
                            start=True, stop=True,
                        )
                    else:
                        nc.tensor.matmul(
                            cps[hi][:], v_sb[:, vcol: vcol + 128],
                            e[:, hi * 512: hi * 512 + 512],
                            start=(kt == 0), stop=(kt == NST - 1),
                        )

            prev = None
            for kt in range(NST):
                blk = hp * S
                if _on("sc") and _CTX_HALVES:
                    # per-hi [128,512] tiles + per-hi exp (baseline-style)
                    scs = [sc_ps.tile([128, 512], F32, tag="sc", name=f"sc_{qc}_{hp}_{kt}_{hi}")
                           for hi in range(2)]
                    es = []
                    for hi in range(2):
                        base = 64 * hi
                        nc.tensor.matmul(
                            scs[hi][:],
                            kT_sb[base:base + 64, blk + kt * 128: blk + kt * 128 + 128],
                            qT_sb[base:base + 64, blk + qc * 512: blk + qc * 512 + 512],
                            start=True, stop=True,
                        )
                        eh = epool.tile([128, 512], BF16 if _CTX_BF16 else F32R,
                                        tag="eh", name=f"e_{qc}_{hp}_{kt}_{hi}")
                        nc.scalar.activation(eh[:], scs[hi][:], EXP, scale=0.125)
                        es.append(eh)
                    e = _HalvesE(es)
                elif _on("sc"):
                    sc = sc_ps.tile([128, 1024], F32, tag="sc", name=f"sc_{qc}_{hp}_{kt}")
                    for hi in range(2):
                        base = 64 * hi
                        kslice = (kT_sb[base:base + 64, 0:128] if _SAME_KT else
                                  kT_sb[base:base + 64, blk + kt * 128: blk + kt * 128 + 128])
                        nc.tensor.matmul(
                            sc[:, hi * 512: hi * 512 + 512],
                            kslice,
                            qT_sb[base:base + 64, blk + qc * 512: blk + qc * 512 + 512],
                            start=True, stop=True,
                        )
                    if _on("exp"):
                        e = epool.tile([128, 1024], BF16 if _CTX_BF16 else F32R,
                                       tag="e", name=f"e_{qc}_{hp}_{kt}")
                        nc.scalar.activation(e[:], sc[:], EXP, scale=0.125)
                    else:
                        e = dummies["e"]
                else:
                    sc = dummies["sc"]
                    if _on("exp"):
                        e = epool.tile([128, 1024], BF16 if _CTX_BF16 else F32R,
                                       tag="e", name=f"e_{qc}_{hp}_{kt}")
                        nc.scalar.activation(e[:], sc[:], EXP, scale=0.125)
                    else:
                        e = dummies["e"]
                if _CTX_NODEP:
                    e = dummies["nodep"]
                if _on("ctx"):
                    if prev is not None:
                        ctx_mms(*prev)
                    prev = (e, kt)
            if _on("ctx"):
                ctx_mms(*prev)

            # normalize: rows 64..127 of each ctx psum tile hold the softmax
            # denominator (replicated by the all-ones V columns)
            for hi in range(2 if _on("ctx") else 0):
                if _CTX_HALVES:
                    tmpa = npool.tile([128, 512], F32, tag="tmpa", name=f"tmp_{qc}_{hp}_{hi}")
                    nc.vector.tensor_copy(tmpa[:], cps[hi][0][:])
                    u = npool.tile([128, 512], F32, tag="u", name=f"u_{qc}_{hp}_{hi}")
                    nc.vector.tensor_add(u[:], cps[hi][1][:], tmpa[:])
                    rec = npool.tile([64, 512], F32, tag="rec", name=f"rec_{qc}_{hp}_{hi}")
                    nc.vector.reciprocal(rec[:], u[64:128, :])
                    nc.vector.tensor_mul(
                        ctxT_sb[64 * hi: 64 * hi + 64, hp * S + qc * 512: hp * S + qc * 512 + 512],
                        u[0:64, :],
                        rec[:],
                    )
                else:
                    rec = npool.tile([64, 512], F32, tag="rec", name=f"rec_{qc}_{hp}_{hi}")
                    nc.vector.reciprocal(rec[:], cps[hi][64:128, :])
                    nc.vector.tensor_mul(
                        ctxT_sb[64 * hi: 64 * hi + 64, hp * S + qc * 512: hp * S + qc * 512 + 512],
                        cps[hi][0:64, :],
                        rec[:],
                    )

        if _ZIGZAG:
            if qc > 1 and _on("out"):
                _out_proj(nc, qc - 2, qk_ps, opool, ctxT_sb, wo_sb, out)
        elif qc > 0 and _on("out"):
            _out_proj(nc, qc - 1, qk_ps, opool, ctxT_sb, wo_sb, out)
    if _ZIGZAG:
        # flush: run the last pass's ctx standalone, then the remaining outprojs
        _zig_pass(nc, None, None, kT_sb, qT_sb, ctxT_sb, v_sb, sc_ps, ctx_ps,
                  epool, npool, zig_state)
        if _on("out"):
            _out_proj(nc, NQC - 2, qk_ps, opool, ctxT_sb, wo_sb, out)
            _out_proj(nc, NQC - 1, qk_ps, opool, ctxT_sb, wo_sb, out)
    elif _on("out"):
        _out_proj(nc, NQC - 1, qk_ps, opool, ctxT_sb, wo_sb, out)


def _zig_pass(nc, qc, hp, kT_sb, qT_sb, ctxT_sb, v_sb, sc_ps, ctx_ps,
              epool, npool, zig_state):
    """One attention pass (qc, hp): issue scores+exp for this pass while
    running the PREVIOUS pass's ctx chains (whose es inputs are long since
    ready, so the PE never waits on ACT).  qc=None flushes the final pass."""
    prev = zig_state["prev"]
    if prev is not None:
        pqc, php, pes = prev
        cps = [
            ctx_ps.tile([128, 512], F32, tag="ctx", name=f"zctx_{pqc}_{php}_{hi}")
            for hi in range(2)
        ]
    else:
        cps = None

    def prev_ctx(kt):
        for hi in range(2):
            h = php * 2 + hi
            vcol = (kt * HPC + h) * 128
            nc.tensor.matmul(
                cps[hi][:], v_sb[:, vcol: vcol + 128],
                pes[kt][:, hi * 512: hi * 512 + 512],
                start=(kt == 0), stop=(kt == NST - 1),
            )

    this_es = []
    if qc is not None:
        blk = hp * S
        for kt in range(NST):
            sc = sc_ps.tile([128, 1024], F32, tag="sc", name=f"zsc_{qc}_{hp}_{kt}")
            for hi in range(2):
                base = 64 * hi
                nc.tensor.matmul(
                    sc[:, hi * 512: hi * 512 + 512],
                    kT_sb[base:base + 64, blk + kt * 128: blk + kt * 128 + 128],
                    qT_sb[base:base + 64, blk + qc * 512: blk + qc * 512 + 512],
                    start=True, stop=True,
                )
            e = epool.tile([128, 1024], BF16, tag="e", name=f"ze_{qc}_{hp}_{kt}")
            nc.scalar.activation(e[:], sc[:], EXP, scale=0.125)
            this_es.append(e)
            if prev is not None:
                prev_ctx(kt)
    elif prev is not None:
        for kt in range(NST):
            prev_ctx(kt)

    if prev is not None:
        for hi in range(2):
            rec = npool.tile([64, 512], F32, tag="rec", name=f"zrec_{pqc}_{php}_{hi}")
            nc.vector.reciprocal(rec[:], cps[hi][64:128, :])
            nc.vector.tensor_mul(
                ctxT_sb[64 * hi: 64 * hi + 64, php * S + pqc * 512: php * S + pqc * 512 + 512],
                cps[hi][0:64, :],
                rec[:],
            )
    zig_state["prev"] = (qc, hp, this_es) if qc is not None else None


def _out_proj(nc, qc, qk_ps, opool, ctxT_sb, wo_sb, out):
    # out projection for this chunk's 4 query tiles
    for qt4 in range(4):
        qt = qc * 4 + qt4
        ot = opool.tile([128, EMB], F32, tag="o", name=f"ot_{qt}")
        for fc in range(2):
            ps = qk_ps.tile([128, 512], F32, tag="qk", name=f"ops_{qt}_{fc}")
            nc.tensor.matmul(
                ps[:],
                ctxT_sb[:, qt * 128: qt * 128 + 128],
                wo_sb[:, fc * 512: fc * 512 + 512],
                start=True, stop=False,
            )
            nc.tensor.matmul(
                ps[:],
                ctxT_sb[:, S + qt * 128: S + qt * 128 + 128],
                wo_sb[:, EMB + fc * 512: EMB + fc * 512 + 512],
                start=False, stop=True,
            )
            nc.vector.tensor_copy(ot[:, fc * 512: fc * 512 + 512], ps[:])
        nc.gpsimd.dma_start(out[qt * 128:(qt + 1) * 128, :], ot[:])


def _build_nc(bench_iters=None):
    from contextlib import ExitStack

    nc = bacc.Bacc("TRN2", target_bir_lowering=False, debug=False, num_devices=NCORES)
    xqT = nc.dram_tensor("xqT", [NQC, 128, KT_E * 512], BF16, kind="ExternalInput").ap()
    xkT = nc.dram_tensor("xkT", [NQC, 128, KT_E * 512], BF16, kind="ExternalInput").ap()
    xvT = nc.dram_tensor("xvT", [NQC, 128, KT_E * 512], BF16, kind="ExternalInput").ap()
    wqT = nc.dram_tensor("wqT", [EMB, DQ], BF16, kind="ExternalInput").ap()
    wkT = nc.dram_tensor("wkT", [EMB, DQ], BF16, kind="ExternalInput").ap()
    wvT = nc.dram_tensor("wvT", [EMB, DQ], BF16, kind="ExternalInput").ap()
    woT = nc.dram_tensor("woT", [DQ, EMB], F32R, kind="ExternalInput").ap()
    bq = nc.dram_tensor("bq", [128, 2], F32, kind="ExternalInput").ap()
    bk = nc.dram_tensor("bk", [128, 2], F32, kind="ExternalInput").ap()
    bv = nc.dram_tensor("bv", [1, DQ], BF16, kind="ExternalInput").ap()
    out = nc.dram_tensor("out", [S, EMB], F32, kind="ExternalOutput").ap()

    with ExitStack() as ctx:
        tc = ctx.enter_context(tile.TileContext(nc))
        _mha(ctx, tc, xqT, xkT, xvT, wqT, wkT, wvT, woT, bq, bk, bv, out,
             bench_iters=bench_iters)
    nc.compile()
    return nc


def _chunk_major(x):
    """[S, EMB] -> x.T as [NQC, 128, KT_E*512]: arr[qc, p, kt*512+m] =
    x.T[kt*128+p, qc*512+m] (one contiguous [128, 4096] DMA per chunk)."""
    xt = np.asarray(x, np.float32).T  # [EMB, S]
    arr = xt.reshape(KT_E, 128, NQC, 512).transpose(2, 1, 0, 3).reshape(NQC, 128, KT_E * 512)
    return np.ascontiguousarray(arr.astype(ml_dtypes.bfloat16))


def make_in_maps(query, key, value, Wq, bq, Wk, bk, Wv, bv, Wo, bo):
    in_maps = []
    for c in range(NCORES):
        b, g = divmod(c, 4)
        rows = slice(g * DQ, (g + 1) * DQ)
        in_maps.append({
            "xqT": _chunk_major(np.asarray(query, np.float32)[b]),
            "xkT": _chunk_major(np.asarray(key, np.float32)[b]),
            "xvT": _chunk_major(np.asarray(value, np.float32)[b]),
            "wqT": np.ascontiguousarray(np.asarray(Wq, np.float32)[rows].T.astype(ml_dtypes.bfloat16)),
            "wkT": np.ascontiguousarray(np.asarray(Wk, np.float32)[rows].T.astype(ml_dtypes.bfloat16)),
            "wvT": np.ascontiguousarray(np.asarray(Wv, np.float32)[rows].T.astype(ml_dtypes.bfloat16)),
            "woT": np.ascontiguousarray(np.asarray(Wo, np.float32)[:, rows].T),
            "bq": np.ascontiguousarray(np.asarray(bq, np.float32)[rows].reshape(2, 128).T),
            "bk": np.ascontiguousarray(np.asarray(bk, np.float32)[rows].reshape(2, 128).T),
            "bv": np.ascontiguousarray(np.asarray(bv, np.float32)[rows][None, :].astype(ml_dtypes.bfloat16)),
        })
    return in_maps


def kernel(query, key, value, Wq, bq, Wk, bk, Wv, bv, Wo, bo):
    global _NC, LAST_RESULT
    bo = np.asarray(bo, dtype=np.float32)
    if _NC is None:
        _NC = _build_nc()

    in_maps = make_in_maps(query, key, value, Wq, bq, Wk, bk, Wv, bv, Wo, bo)

    res = bass_utils.run_bass_kernel_spmd(
        _NC, in_maps, core_ids=list(range(NCORES)), trace=TRACE
    )
    LAST_RESULT = res

    out = np.zeros((B, S, EMB), np.float32)
    for c in range(NCORES):
        out[c // 4] += res.results[c]["out"]
    out += bo[None, None, :]
    return out


# revision 22
# speedup vs baseline: 1.2435x; 1.0306x over previous
"""Multi-head attention (B=2, S=2048, EMB=1024, 16 heads) on 8 Trainium2 cores.

Sharding: core c -> batch c//4, head-group c%4 (4 heads = 256 projection dims).
Each core computes Q/K/V projections for its head group in transposed layout
(Q^T, K^T with head-dim on partitions; V natural), attention without max
subtraction (scores ~ N(0,1), exp never overflows in fp32), and a row-parallel
partial of the output projection.  The host sums the 4 partials per batch and
adds the output bias (the all-reduce of the row-parallel fc_out is done at
unshard time; no device collectives needed).

Key PE-efficiency choices (PE issue time = moving-dim cycles per matmul,
independent of contraction rows):
  - ctx accumulation runs one K=128 chain per head (not two K=64 halves).
  - The stationary V block per (k-tile, head) is [128, 128]: cols 0..63 the
    head's V, cols 64..127 all-ones.  Out rows 64..127 then all hold the
    softmax denominator -- replicated across partitions for free by the PE
    (those array columns were idle anyway), so normalization is just a
    reciprocal + multiply on DVE with no broadcast step.
  - Q/K biases are folded into the PSUM->SBUF drain (tensor_scalar_add);
    V bias stays as a 1-row matmul (free-dim bias can't ride the drain).
  - exp runs one ACT instruction per [128, 1024] PSUM tile (a head-pair's
    scores for one k-tile), halving ACT per-instruction overhead.
  - K projections for all chunks run first; V projections are fused into the
    first attention pass (V is not consumed until the second pass's ctx), so
    the ACT exp stream starts as early as possible.
  - Attention passes are zigzag-pipelined: pass p's scores+exp (PE->ACT) run
    concurrently with pass p-1's ctx chains, whose es inputs (bf16, SBUF ring
    across two passes) are long since ready -- the PE never waits on ACT.

All matmuls run as float32r (full-rate on the PE at N>=256).  The BIR verifier
requires fp32r matmul operands to be *produced* as fp32r, so every
matmul-feeding tensor is declared fp32r end-to-end (same 4-byte layout as
fp32; host passes float32 arrays).
"""

import ml_dtypes
import numpy as np

import concourse.tile as tile
from concourse import bacc, mybir
from concourse import bass_utils

EMB = 1024
S = 2048
B = 2
HPC = 4            # heads per core
DQ = HPC * 64      # 256 projection dims per core
NCORES = 8

F32 = mybir.dt.float32
F32R = mybir.dt.float32r
BF16 = mybir.dt.bfloat16
EXP = mybir.ActivationFunctionType.Exp

KT_E = EMB // 128  # 8 contraction tiles over EMB
NQC = S // 512     # 4 query chunks
NST = S // 128     # 16 sequence tiles

_NC = None
TRACE = False
LAST_RESULT = None
_SMALL_DMA = False
_PHASES = None   # None = all; else subset of {"proj","sc","exp","ctx","out"}
_SAME_KT = False # bench probe: all scores matmuls reuse one stationary block
_CTX_BF16 = False  # es + v in bf16 (correct kernel; probes fp32r accum rate)
_CTX_NODEP = False # bench probe: ctx rhs is a constant tile (no ACT dep)
_CTX_ALLSTART = False  # bench probe: ctx matmuls all start/stop (no accumulate)
_CTX_HALVES = False    # ctx as two row-disjoint K=64 half chains (correct)
_ZIGZAG = True   # ctx chains of pass p-1 run against scores+exp of pass p
_VFUSE = True    # emit V projections inside the first attention pass


def _on(ph):
    return _PHASES is None or ph in _PHASES


def _mha(ctx, tc, xqT, xkT, xvT, wqT, wkT, wvT, woT, bq, bk, bv, out, bench_iters=None):
    nc = tc.nc

    cpool = ctx.enter_context(tc.tile_pool(name="const", bufs=1))
    xpool = ctx.enter_context(tc.tile_pool(name="xin", bufs=4))
    epool = ctx.enter_context(tc.tile_pool(name="exp", bufs=34 if _ZIGZAG else 4))
    npool = ctx.enter_context(tc.tile_pool(name="norm", bufs=4))
    opool = ctx.enter_context(tc.tile_pool(name="osb", bufs=3))
    qk_ps = ctx.enter_context(tc.tile_pool(name="qkps", bufs=1 if _CTX_HALVES else 2, space="PSUM"))
    sc_ps = ctx.enter_context(tc.tile_pool(name="scps", bufs=3 if _CTX_HALVES else 2, space="PSUM"))
    ctx_ps = ctx.enter_context(tc.tile_pool(name="ctxps", bufs=4 if _CTX_HALVES else 2, space="PSUM"))

    # ---- persistent SBUF tensors ----
    # K/V weights first: phase 1 needs them before anything else.
    ones_row = cpool.tile([1, 512], BF16)
    nc.vector.memset(ones_row[:], 1.0)

    wq_sb = cpool.tile([128, KT_E * DQ], BF16)  # [128, 2048]: wq_sb[p, n*256+m] = WqT[n*128+p, m]
    wk_sb = cpool.tile([128, KT_E * DQ], BF16)
    wv_sb = cpool.tile([128, KT_E * DQ], BF16)
    for sb, src in ((wk_sb, wkT), (wv_sb, wvT), (wq_sb, wqT)):
        nc.scalar.dma_start(
            sb[:].rearrange("p (n m) -> p n m", n=KT_E),
            src.rearrange("(n p) m -> p n m", p=128),
        )
    bq_sb = cpool.tile([128, 2], F32)           # bq_sb[p, dq] = bq[dq*128+p]
    bk_sb = cpool.tile([128, 2], F32)
    for sb, src in ((bq_sb, bq), (bk_sb, bk)):
        nc.scalar.dma_start(sb[:], src[:])
    bv_sb = cpool.tile([1, DQ], BF16)
    nc.scalar.dma_start(bv_sb[:], bv[:])
    wo_sb = cpool.tile([128, 2 * EMB], F32R)    # wo_sb[p, n*1024+f] = WoT[n*128+p, f]
    nc.scalar.dma_start(
        wo_sb[:].rearrange("p (n m) -> p n m", n=2),
        woT.rearrange("(n p) m -> p n m", p=128),
    )

    # results of the projection phases kept resident
    kT_sb = cpool.tile([128, 2 * S], F32R)      # [dq-block 2][s 2048]
    qT_sb = cpool.tile([128, 2 * S], F32R)
    ctxT_sb = cpool.tile([128, 2 * S], F32R)
    # per (k-tile, head) block of 128 cols: [0:64] = V, [64:128] = ones
    v_dt = BF16 if (_CTX_BF16 or _ZIGZAG) else F32R
    v_sb = cpool.tile([128, NST * (HPC * 128)], v_dt)
    v_ones = v_sb[:].rearrange("p (t h m) -> p t h m", t=NST, h=HPC)[:, :, :, 64:128]
    nc.vector.memset(v_ones if v_dt == BF16 else v_ones.bitcast(F32), 1.0)

    dummies = {}
    if _PHASES is not None:
        for t in (kT_sb, qT_sb, ctxT_sb, v_sb):
            nc.vector.memset(t[:].bitcast(F32), 0.001)
        if not _on("sc"):
            dsc = sc_ps.tile([128, 1024], F32, tag="sc", name="dummy_sc")
            nc.vector.memset(dsc[:], 0.001)
            dummies["sc"] = dsc
        if not _on("exp"):
            de = epool.tile([128, 1024], F32R, tag="e", name="dummy_e")
            nc.vector.memset(de[:].bitcast(F32), 0.001)
            dummies["e"] = de
    if _CTX_NODEP:
        dn = epool.tile([128, 1024], BF16 if _CTX_BF16 else F32R, tag="e", name="dummy_nodep")
        nc.vector.memset(dn[:] if _CTX_BF16 else dn[:].bitcast(F32), 0.001)
        dummies["nodep"] = dn

    def body():
        _body(tc, nc, xqT, xkT, xvT, out, ones_row, wq_sb, wk_sb,
              wv_sb, wo_sb, bq_sb, bk_sb, bv_sb, kT_sb, qT_sb, ctxT_sb, v_sb,
              xpool, epool, npool, opool, qk_ps, sc_ps, ctx_ps, dummies)

    if bench_iters:
        hints = (
            mybir.EngineType.PE,
            mybir.EngineType.Activation,
            mybir.EngineType.DVE,
            mybir.EngineType.SP,
            mybir.EngineType.Pool,
        )
        with tc.For_i(0, bench_iters, 1, hint_engines=hints):
            body()
    else:
        body()


class _HalvesE:
    def __init__(self, es):
        self.es = es

    def __getitem__(self, idx):
        rows, cols = idx
        hi = cols.start // 512
        off = cols.start - hi * 512
        return self.es[hi][rows, off: off + (cols.stop - cols.start)]


def _proj_chain(nc, ps, w_sb, xs, dq):
    """Q/K projection chain into psum ps: kt=0 full K=128 (group start, marks
    every element written), then 7 more accumulating K=128 steps."""
    col = lambda kt: kt * DQ + dq * 128
    for kt in range(KT_E):
        nc.tensor.matmul(
            ps[:], w_sb[:, col(kt): col(kt) + 128], xs[kt],
            start=(kt == 0), stop=(kt == KT_E - 1),
        )


def _body(tc, nc, xqT, xkT, xvT, out, ones_row, wq_sb, wk_sb,
          wv_sb, wo_sb, bq_sb, bk_sb, bv_sb, kT_sb, qT_sb, ctxT_sb, v_sb,
          xpool, epool, npool, opool, qk_ps, sc_ps, ctx_ps, dummies=None):
    # ---- phase 1: K and V projections for every chunk (Q deferred so the
    # attention pipeline -- and with it the ACT exp stream -- starts as soon
    # as K/V are resident).
    for qc in range(NQC if _on("proj") else 0):
        xkb = xpool.tile([128, KT_E * 512], BF16, tag="xchunk", name=f"xk_{qc}")
        if _SMALL_DMA:
            for kt in range(KT_E):
                nc.sync.dma_start(xkb[:, kt * 512: kt * 512 + 512],
                                  xkT[qc][:, kt * 512: kt * 512 + 512])
        else:
            nc.sync.dma_start(xkb[:], xkT[qc])
        xk = [xkb[:, kt * 512: kt * 512 + 512] for kt in range(KT_E)]
        for dq in range(2):
            ps = qk_ps.tile([128, 512], F32, tag="qk", name=f"kps_{qc}_{dq}")
            _proj_chain(nc, ps, wk_sb, xk, dq)
            nc.vector.tensor_scalar_add(
                kT_sb[:, dq * S + qc * 512: dq * S + qc * 512 + 512],
                ps[:], bk_sb[:, dq: dq + 1],
            )
        if _VFUSE:
            continue
        xvb = xpool.tile([128, KT_E * 512], BF16, tag="xchunk", name=f"xv_{qc}")
        if _SMALL_DMA:
            for kt in range(KT_E):
                nc.sync.dma_start(xvb[:, kt * 512: kt * 512 + 512],
                                  xvT[qc][:, kt * 512: kt * 512 + 512])
        else:
            nc.sync.dma_start(xvb[:], xvT[qc])
        xv = [xvb[:, kt * 512: kt * 512 + 512] for kt in range(KT_E)]
        for sti in range(4):
            st = qc * 4 + sti
            if _CTX_HALVES:
                ps = qk_ps.tile([128, 256], F32, tag="qk", name=f"vps_{st}")
            else:
                ps = sc_ps.tile([128, 256], F32, tag="sc", name=f"vps_{st}")
            for kt in range(KT_E):
                nc.tensor.matmul(
                    ps[:], xv[kt][:, sti * 128: sti * 128 + 128],
                    wv_sb[:, kt * DQ: kt * DQ + DQ],
                    start=(kt == 0), stop=False,
                )
            nc.tensor.matmul(
                ps[:], ones_row[0:1, 0:128], bv_sb[0:1, :],
                start=False, stop=True,
            )
            dst = v_sb[:, st * (HPC * 128): (st + 1) * (HPC * 128)]
            nc.vector.tensor_copy(
                dst.rearrange("p (h m) -> p h m", h=HPC)[:, :, 0:64],
                ps[:].rearrange("p (h m) -> p h m", h=HPC),
            )

    # ---- phase 2: per query chunk: Q projection, attention, then the
    # previous chunk's output projection (deferred one qc so it overlaps
    # with this chunk's ACT-bound attention).
    zig_state = {"prev": None}
    if _VFUSE:
        zig_state["vwork"] = _make_vwork(
            nc, xvT, xpool, qk_ps, v_sb, wv_sb, bv_sb, ones_row)
    for qc in range(NQC):
      if _on("proj"):
        xqb = xpool.tile([128, KT_E * 512], BF16, tag="xchunk", name=f"xq_{qc}")
        if _SMALL_DMA:
            for kt in range(KT_E):
                nc.sync.dma_start(xqb[:, kt * 512: kt * 512 + 512],
                                  xqT[qc][:, kt * 512: kt * 512 + 512])
        else:
            nc.sync.dma_start(xqb[:], xqT[qc])
        xq = [xqb[:, kt * 512: kt * 512 + 512] for kt in range(KT_E)]
        for dq in range(2):
            ps = qk_ps.tile([128, 512], F32, tag="qk", name=f"qps_{qc}_{dq}")
            _proj_chain(nc, ps, wq_sb, xq, dq)
            nc.vector.tensor_scalar_add(
                qT_sb[:, dq * S + qc * 512: dq * S + qc * 512 + 512],
                ps[:], bq_sb[:, dq: dq + 1],
            )

        # attention, two heads at a time (the head pair shares one
        # [128, 1024] score tile -> one exp instruction per k-tile)
        for hp in range(HPC // 2):
            if _ZIGZAG:
                _zig_pass(nc, qc, hp, kT_sb, qT_sb, ctxT_sb, v_sb, sc_ps, ctx_ps,
                          epool, npool, zig_state)
                continue
            if _on("ctx") and _CTX_HALVES:
                cps = [
                    [ctx_ps.tile([128, 512], F32, tag="ctx", name=f"ctx_{qc}_{hp}_{hi}_{ha}")
                     for ha in range(2)]
                    for hi in range(2)
                ]
            elif _on("ctx"):
                cps = [
                    ctx_ps.tile([128, 512], F32, tag="ctx", name=f"ctx_{qc}_{hp}_{hi}")
                    for hi in range(2)
                ]
            else:
                cps = None

            def ctx_mms(e, kt):
                for hi in range(2):
                    h = hp * 2 + hi
                    vcol = (kt * HPC + h) * 128
                    if _CTX_HALVES:
                        for ha, b in enumerate((0, 64)):
                            nc.tensor.matmul(
                                cps[hi][ha][:], v_sb[b:b + 64, vcol: vcol + 128],
                                e[b:b + 64, hi * 512: hi * 512 + 512],
                                start=(kt == 0), stop=(kt == NST - 1),
                            )
                    elif _CTX_ALLSTART:
                        nc.tensor.matmul(
                            cps[hi][:], v_sb[:, vcol: vcol + 128],
                            e[:, hi * 512: hi * 512 + 512],
                            start=True, stop=True,
                        )
                    else:
                        nc.tensor.matmul(
                            cps[hi][:], v_sb[:, vcol: vcol + 128],
                            e[:, hi * 512: hi * 512 + 512],
                            start=(kt == 0), stop=(kt == NST - 1),
                        )

            prev = None
            for kt in range(NST):
                blk = hp * S
                if _on("sc") and _CTX_HALVES:
                    # per-hi [128,512] tiles + per-hi exp (baseline-style)
                    scs = [sc_ps.tile([128, 512], F32, tag="sc", name=f"sc_{qc}_{hp}_{kt}_{hi}")
                           for hi in range(2)]
                    es = []
                    for hi in range(2):
                        base = 64 * hi
                        nc.tensor.matmul(
                            scs[hi][:],
                            kT_sb[base:base + 64, blk + kt * 128: blk + kt * 128 + 128],
                            qT_sb[base:base + 64, blk + qc * 512: blk + qc * 512 + 512],
                            start=True, stop=True,
                        )
                        eh = epool.tile([128, 512], BF16 if _CTX_BF16 else F32R,
                                        tag="eh", name=f"e_{qc}_{hp}_{kt}_{hi}")
                        nc.scalar.activation(eh[:], scs[hi][:], EXP, scale=0.125)
                        es.append(eh)
                    e = _HalvesE(es)
                elif _on("sc"):
                    sc = sc_ps.tile([128, 1024], F32, tag="sc", name=f"sc_{qc}_{hp}_{kt}")
                    for hi in range(2):
                        base = 64 * hi
                        kslice = (kT_sb[base:base + 64, 0:128] if _SAME_KT else
                                  kT_sb[base:base + 64, blk + kt * 128: blk + kt * 128 + 128])
                        nc.tensor.matmul(
                            sc[:, hi * 512: hi * 512 + 512],
                            kslice,
                            qT_sb[base:base + 64, blk + qc * 512: blk + qc * 512 + 512],
                            start=True, stop=True,
                        )
                    if _on("exp"):
                        e = epool.tile([128, 1024], BF16 if _CTX_BF16 else F32R,
                                       tag="e", name=f"e_{qc}_{hp}_{kt}")
                        nc.scalar.activation(e[:], sc[:], EXP, scale=0.125)
                    else:
                        e = dummies["e"]
                else:
                    sc = dummies["sc"]
                    if _on("exp"):
                        e = epool.tile([128, 1024], BF16 if _CTX_BF16 else F32R,
                                       tag="e", name=f"e_{qc}_{hp}_{kt}")
                        nc.scalar.activation(e[:], sc[:], EXP, scale=0.125)
                    else:
                        e = dummies["e"]
                if _CTX_NODEP:
                    e = dummies["nodep"]
                if _on("ctx"):
                    if prev is not None:
                        ctx_mms(*prev)
                    prev = (e, kt)
            if _on("ctx"):
                ctx_mms(*prev)

            # normalize: rows 64..127 of each ctx psum tile hold the softmax
            # denominator (replicated by the all-ones V columns)
            for hi in range(2 if _on("ctx") else 0):
                if _CTX_HALVES:
                    tmpa = npool.tile([128, 512], F32, tag="tmpa", name=f"tmp_{qc}_{hp}_{hi}")
                    nc.vector.tensor_copy(tmpa[:], cps[hi][0][:])
                    u = npool.tile([128, 512], F32, tag="u", name=f"u_{qc}_{hp}_{hi}")
                    nc.vector.tensor_add(u[:], cps[hi][1][:], tmpa[:])
                    rec = npool.tile([64, 512], F32, tag="rec", name=f"rec_{qc}_{hp}_{hi}")
                    nc.vector.reciprocal(rec[:], u[64:128, :])
                    nc.vector.tensor_mul(
                        ctxT_sb[64 * hi: 64 * hi + 64, hp * S + qc * 512: hp * S + qc * 512 + 512],
                        u[0:64, :],
                        rec[:],
                    )
                else:
                    rec = npool.tile([64, 512], F32, tag="rec", name=f"rec_{qc}_{hp}_{hi}")
                    nc.vector.reciprocal(rec[:], cps[hi][64:128, :])
                    nc.vector.tensor_mul(
                        ctxT_sb[64 * hi: 64 * hi + 64, hp * S + qc * 512: hp * S + qc * 512 + 512],
                        cps[hi][0:64, :],
                        rec[:],
                    )

        if _ZIGZAG:
            if qc > 1 and _on("out"):
                _out_proj(nc, qc - 2, qk_ps, opool, ctxT_sb, wo_sb, out)
        elif qc > 0 and _on("out"):
            _out_proj(nc, qc - 1, qk_ps, opool, ctxT_sb, wo_sb, out)
    if _ZIGZAG:
        # flush: run the last pass's ctx standalone, then the remaining outprojs
        _zig_pass(nc, None, None, kT_sb, qT_sb, ctxT_sb, v_sb, sc_ps, ctx_ps,
                  epool, npool, zig_state)
        if _on("out"):
            _out_proj(nc, NQC - 2, qk_ps, opool, ctxT_sb, wo_sb, out)
            _out_proj(nc, NQC - 1, qk_ps, opool, ctxT_sb, wo_sb, out)
    elif _on("out"):
        _out_proj(nc, NQC - 1, qk_ps, opool, ctxT_sb, wo_sb, out)


def _make_vwork(nc, xvT, xpool, qk_ps, v_sb, wv_sb, bv_sb, ones_row):
    """Return a list of 16 closures, one per sequence tile st, each emitting
    that tile's V projection (chain + bias + drain).  DMAs are issued lazily
    per chunk on first use."""
    state = {"xv": {}}

    def xv_chunk(vc):
        if vc not in state["xv"]:
            xvb = xpool.tile([128, KT_E * 512], BF16, tag="xchunk", name=f"xv_{vc}")
            nc.sync.dma_start(xvb[:], xvT[vc])
            state["xv"][vc] = xvb
        return state["xv"][vc]

    def make(st):
        def emit():
            vc, sti = divmod(st, 4)
            xvb = xv_chunk(vc)
            xv = [xvb[:, kt * 512: kt * 512 + 512] for kt in range(KT_E)]
            ps = qk_ps.tile([128, 256], F32, tag="qk", name=f"vps_{st}")
            for kt in range(KT_E):
                nc.tensor.matmul(
                    ps[:], xv[kt][:, sti * 128: sti * 128 + 128],
                    wv_sb[:, kt * DQ: kt * DQ + DQ],
                    start=(kt == 0), stop=False,
                )
            nc.tensor.matmul(
                ps[:], ones_row[0:1, 0:128], bv_sb[0:1, :],
                start=False, stop=True,
            )
            dst = v_sb[:, st * (HPC * 128): (st + 1) * (HPC * 128)]
            nc.vector.tensor_copy(
                dst.rearrange("p (h m) -> p h m", h=HPC)[:, :, 0:64],
                ps[:].rearrange("p (h m) -> p h m", h=HPC),
            )
        return emit

    return [make(st) for st in range(NST)]


def _zig_pass(nc, qc, hp, kT_sb, qT_sb, ctxT_sb, v_sb, sc_ps, ctx_ps,
              epool, npool, zig_state):
    """One attention pass (qc, hp): issue scores+exp for this pass while
    running the PREVIOUS pass's ctx chains (whose es inputs are long since
    ready, so the PE never waits on ACT).  qc=None flushes the final pass."""
    prev = zig_state["prev"]
    if prev is not None:
        pqc, php, pes = prev
        cps = [
            ctx_ps.tile([128, 512], F32, tag="ctx", name=f"zctx_{pqc}_{php}_{hi}")
            for hi in range(2)
        ]
    else:
        cps = None

    def prev_ctx(kt):
        for hi in range(2):
            h = php * 2 + hi
            vcol = (kt * HPC + h) * 128
            nc.tensor.matmul(
                cps[hi][:], v_sb[:, vcol: vcol + 128],
                pes[kt][:, hi * 512: hi * 512 + 512],
                start=(kt == 0), stop=(kt == NST - 1),
            )

    this_es = []
    if qc is not None:
        blk = hp * S
        for kt in range(NST):
            sc = sc_ps.tile([128, 1024], F32, tag="sc", name=f"zsc_{qc}_{hp}_{kt}")
            for hi in range(2):
                base = 64 * hi
                nc.tensor.matmul(
                    sc[:, hi * 512: hi * 512 + 512],
                    kT_sb[base:base + 64, blk + kt * 128: blk + kt * 128 + 128],
                    qT_sb[base:base + 64, blk + qc * 512: blk + qc * 512 + 512],
                    start=True, stop=True,
                )
            e = epool.tile([128, 1024], BF16, tag="e", name=f"ze_{qc}_{hp}_{kt}")
            nc.scalar.activation(e[:], sc[:], EXP, scale=0.125)
            this_es.append(e)
            vwork = zig_state.get("vwork")
            if vwork:
                vwork.pop(0)()
            if prev is not None:
                prev_ctx(kt)
    elif prev is not None:
        for kt in range(NST):
            prev_ctx(kt)

    if prev is not None:
        for hi in range(2):
            rec = npool.tile([64, 512], F32, tag="rec", name=f"zrec_{pqc}_{php}_{hi}")
            nc.vector.reciprocal(rec[:], cps[hi][64:128, :])
            nc.vector.tensor_mul(
                ctxT_sb[64 * hi: 64 * hi + 64, php * S + pqc * 512: php * S + pqc * 512 + 512],
                cps[hi][0:64, :],
                rec[:],
            )
    zig_state["prev"] = (qc, hp, this_es) if qc is not None else None


def _out_proj(nc, qc, qk_ps, opool, ctxT_sb, wo_sb, out):
    # out projection for this chunk's 4 query tiles
    for qt4 in range(4):
        qt = qc * 4 + qt4
        ot = opool.tile([128, EMB], F32, tag="o", name=f"ot_{qt}")
        for fc in range(2):
            ps = qk_ps.tile([128, 512], F32, tag="qk", name=f"ops_{qt}_{fc}")
            nc.tensor.matmul(
                ps[:],
                ctxT_sb[:, qt * 128: qt * 128 + 128],
                wo_sb[:, fc * 512: fc * 512 + 512],
                start=True, stop=False,
            )
            nc.tensor.matmul(
                ps[:],
                ctxT_sb[:, S + qt * 128: S + qt * 128 + 128],
                wo_sb[:, EMB + fc * 512: EMB + fc * 512 + 512],
                start=False, stop=True,
            )
            nc.vector.tensor_copy(ot[:, fc * 512: fc * 512 + 512], ps[:])
        nc.gpsimd.dma_start(out[qt * 128:(qt + 1) * 128, :], ot[:])


def _build_nc(bench_iters=None):
    from contextlib import ExitStack

    nc = bacc.Bacc("TRN2", target_bir_lowering=False, debug=False, num_devices=NCORES)
    xqT = nc.dram_tensor("xqT", [NQC, 128, KT_E * 512], BF16, kind="ExternalInput").ap()
    xkT = nc.dram_tensor("xkT", [NQC, 128, KT_E * 512], BF16, kind="ExternalInput").ap()
    xvT = nc.dram_tensor("xvT", [NQC, 128, KT_E * 512], BF16, kind="ExternalInput").ap()
    wqT = nc.dram_tensor("wqT", [EMB, DQ], BF16, kind="ExternalInput").ap()
    wkT = nc.dram_tensor("wkT", [EMB, DQ], BF16, kind="ExternalInput").ap()
    wvT = nc.dram_tensor("wvT", [EMB, DQ], BF16, kind="ExternalInput").ap()
    woT = nc.dram_tensor("woT", [DQ, EMB], F32R, kind="ExternalInput").ap()
    bq = nc.dram_tensor("bq", [128, 2], F32, kind="ExternalInput").ap()
    bk = nc.dram_tensor("bk", [128, 2], F32, kind="ExternalInput").ap()
    bv = nc.dram_tensor("bv", [1, DQ], BF16, kind="ExternalInput").ap()
    out = nc.dram_tensor("out", [S, EMB], F32, kind="ExternalOutput").ap()

    with ExitStack() as ctx:
        tc = ctx.enter_context(tile.TileContext(nc))
        _mha(ctx, tc, xqT, xkT, xvT, wqT, wkT, wvT, woT, bq, bk, bv, out,
             bench_iters=bench_iters)
    nc.compile()
    return nc


def _chunk_major(x):
    """[S, EMB] -> x.T as [NQC, 128, KT_E*512]: arr[qc, p, kt*512+m] =
    x.T[kt*128+p, qc*512+m] (one contiguous [128, 4096] DMA per chunk)."""
    xt = np.asarray(x, np.float32).T  # [EMB, S]
    arr = xt.reshape(KT_E, 128, NQC, 512).transpose(2, 1, 0, 3).reshape(NQC, 128, KT_E * 512)
    return np.ascontiguousarray(arr.astype(ml_dtypes.bfloat16))


def make_in_maps(query, key, value, Wq, bq, Wk, bk, Wv, bv, Wo, bo):
    in_maps = []
    for c in range(NCORES):
        b, g = divmod(c, 4)
        rows = slice(g * DQ, (g + 1) * DQ)
        in_maps.append({
            "xqT": _chunk_major(np.asarray(query, np.float32)[b]),
            "xkT": _chunk_major(np.asarray(key, np.float32)[b]),
            "xvT": _chunk_major(np.asarray(value, np.float32)[b]),
            "wqT": np.ascontiguousarray(np.asarray(Wq, np.float32)[rows].T.astype(ml_dtypes.bfloat16)),
            "wkT": np.ascontiguousarray(np.asarray(Wk, np.float32)[rows].T.astype(ml_dtypes.bfloat16)),
            "wvT": np.ascontiguousarray(np.asarray(Wv, np.float32)[rows].T.astype(ml_dtypes.bfloat16)),
            "woT": np.ascontiguousarray(np.asarray(Wo, np.float32)[:, rows].T),
            "bq": np.ascontiguousarray(np.asarray(bq, np.float32)[rows].reshape(2, 128).T),
            "bk": np.ascontiguousarray(np.asarray(bk, np.float32)[rows].reshape(2, 128).T),
            "bv": np.ascontiguousarray(np.asarray(bv, np.float32)[rows][None, :].astype(ml_dtypes.bfloat16)),
        })
    return in_maps


def kernel(query, key, value, Wq, bq, Wk, bk, Wv, bv, Wo, bo):
    global _NC, LAST_RESULT
    bo = np.asarray(bo, dtype=np.float32)
    if _NC is None:
        _NC = _build_nc()

    in_maps = make_in_maps(query, key, value, Wq, bq, Wk, bk, Wv, bv, Wo, bo)

    res = bass_utils.run_bass_kernel_spmd(
        _NC, in_maps, core_ids=list(range(NCORES)), trace=TRACE
    )
    LAST_RESULT = res

    out = np.zeros((B, S, EMB), np.float32)
    for c in range(NCORES):
        out[c // 4] += res.results[c]["out"]
    out += bo[None, None, :]
    return out
